# revision 3
# baseline (speedup 1.0000x reference)
"""CIN (xDeepFM) 3-layer kernel for Trainium2, 8-core data parallel.

Math (per layer l, with IN = input viewed [F=64, n] and X = previous
activation [H, n], n = (b, d) flattened):
    pre[o, n] = sum_{h, f} Wl[o, h, f] * X[h, n] * IN[f, n]
    Xnext = relu(pre + bl);  out_l[o, b] = sum_d Xnext[o, (b, d)]

Device strategy per core (64 batches, n = 0..2047):
  - Layer 0: z0[(h,f), n] = IN[h,n]*IN[f,n] is symmetric in (h,f), so W0 is
    folded to upper-triangle form on host (K 4096 -> 2080, padded 2176) and
    the z0 rows themselves are host-packed and DMA'd in (they are a pure
    function of the input, like im2col). No on-device elementwise work for
    layer 0, and only 17 K=128 matmul tiles.
  - Tables t[p, j, n] = IN[8g+j, n] are partition-broadcast by the DMA
    itself (stride-0 source over partitions): HBM reads are ~64KB instead
    of 33MB; only the SBUF-write side pays.
  - Layers 1/2: z k-tiles (8 f-rows x 128 h) built by elementwise
    tensor_mul: 7 tiles per layer on VectorE (2 elem/cycle bf16), 1 tile on
    GpSimd. The GpSimd tile is issued first after each relu (it is ~4x
    slower per tile) and its matmuls are accumulated last.
  - Matmuls in bf16 accumulate over f into one PSUM bank [o=128, 512].
  - ScalarE applies bias+ReLU straight from PSUM (per-partition bias),
    producing the next layer's X in its natural layout. No transposes.
  - VectorE grouped reduce sums over d (innermost 32) for the output.
"""

import numpy as np
import ml_dtypes

import concourse.bass as bass
import concourse.bacc as bacc
import concourse.tile as tile
import concourse.mybir as mybir
from concourse.bass_utils import run_bass_kernel_spmd

BF16 = ml_dtypes.bfloat16

B, F, D = 512, 64, 32
NCORES = 8
BL = B // NCORES          # 64 batches per core
N = BL * D                # 2048 columns per core
CH = 512                  # chunk width (columns)
NCH = N // CH             # 4 chunks
O = 128                   # out channels per layer
GRP = 8                   # f-rows per z tile
NP = 2080                 # symmetric (h<=f) pairs for layer 0
K0T = 17                  # ceil(2080/128) k-tiles for layer 0
GPT = 1                   # z tiles per layer built on GpSimd (of 8)
bf16 = mybir.dt.bfloat16
f32 = mybir.dt.float32

_cache = {}


def _build_program(bench_repeat=None, gpt=GPT):
    from contextlib import ExitStack, nullcontext

    nc = bacc.Bacc("TRN2")
    ins = nc.declare_dram_parameter("ins", [NCH, GRP, GRP, CH], bf16,
                                    isOutput=False)
    z0 = nc.declare_dram_parameter("z0", [NCH, 128, K0T, CH], bf16,
                                   isOutput=False)
    w0 = nc.declare_dram_parameter("w0", [128, K0T, 128], bf16, isOutput=False)
    w1 = nc.declare_dram_parameter("w1", [128, 64, 128], bf16, isOutput=False)
    w2 = nc.declare_dram_parameter("w2", [128, 64, 128], bf16, isOutput=False)
    b0 = nc.declare_dram_parameter("b0", [128, 1], f32, isOutput=False)
    b1 = nc.declare_dram_parameter("b1", [128, 1], f32, isOutput=False)
    b2 = nc.declare_dram_parameter("b2", [128, 1], f32, isOutput=False)
    out = nc.declare_dram_parameter("out", [3, 128, BL], f32, isOutput=True)

    with tile.TileContext(nc) as tc, ExitStack() as ctx:
        wpool = ctx.enter_context(tc.tile_pool(name="w", bufs=1))
        z0pool = ctx.enter_context(tc.tile_pool(name="z0", bufs=2))
        xc_pool = ctx.enter_context(tc.tile_pool(name="xc", bufs=4))
        tabs = ctx.enter_context(tc.tile_pool(name="tabs", bufs=11))
        zv_pool = ctx.enter_context(tc.tile_pool(name="zv", bufs=3))
        zg_pool = ctx.enter_context(tc.tile_pool(name="zg", bufs=2))
        opool = ctx.enter_context(tc.tile_pool(name="oacc", bufs=1))
        pspool = ctx.enter_context(tc.tile_pool(name="ps", bufs=4, space="PSUM"))

        # resident weights [p=k-row, f-slot, o]
        w0_t = wpool.tile([128, K0T, 128], bf16)
        nc.sync.dma_start(w0_t[:], w0[:])
        w1_t = wpool.tile([128, 64, 128], bf16)
        nc.sync.dma_start(w1_t[:], w1[:])
        w2_t = wpool.tile([128, 64, 128], bf16)
        nc.sync.dma_start(w2_t[:], w2[:])
        bias_ts = []
        for bi, bp in enumerate((b0, b1, b2)):
            b_t = wpool.tile([128, 1], f32, name=f"bias{bi}", tag=f"bias{bi}")
            nc.sync.dma_start(b_t[:], bp[:])
            bias_ts.append(b_t)

        oacc = [opool.tile([128, BL], f32, name=f"oacc{i}", tag=f"oacc{i}")
                for i in range(3)]

        loop_cm = tc.For_i(0, bench_repeat, 1) if bench_repeat else nullcontext()
        with loop_cm:
          for c in range(NCH):
              ns = c * CH

              # ---- stream inputs for this chunk
              z0_t = z0pool.tile([128, K0T, CH], bf16, tag="z0")
              nc.sync.dma_start(z0_t[:], z0[c])
              t1 = []
              for g in range(GRP):
                  s = tabs.tile([128, GRP, CH], bf16, tag="tab")
                  nc.sync.dma_start(
                      s[:], ins[c, g].unsqueeze(0).broadcast_to([128, GRP, CH]))
                  t1.append(s)

              # ---- layer 0: 17 K=128 tiles from the host-packed symmetric z0
              ps0 = pspool.tile([128, CH], f32)
              for s in range(K0T):
                  nc.tensor.matmul(ps0[:], w0_t[:, s, :], z0_t[:, s, :],
                                   start=(s == 0), stop=(s == K0T - 1))
              x1c = xc_pool.tile([128, CH], bf16, tag="xc")
              nc.scalar.activation(x1c[:], ps0[:],
                                   mybir.ActivationFunctionType.Relu,
                                   bias=bias_ts[0], scale=1.0)
              nc.vector.tensor_reduce(
                  oacc[0][:, c * (CH // D):(c + 1) * (CH // D)],
                  x1c.rearrange("p (g d) -> p g d", d=D),
                  axis=mybir.AxisListType.X, op=mybir.AluOpType.add)

              # ---- layers 1 and 2: GpSimd builds the first `gpt` z tiles,
              # VectorE the rest; matmuls consume VectorE tiles first.
              xin = x1c
              for li, (w_t, ps_i) in enumerate(((w1_t, 1), (w2_t, 2))):
                  ps = pspool.tile([128, CH], f32)
                  zg_tiles = []
                  for g in range(gpt):
                      zgt = zg_pool.tile([128, GRP, CH], bf16, tag="zg")
                      nc.gpsimd.tensor_mul(
                          zgt[:], xin.unsqueeze(1)
                          .broadcast_to([128, GRP, CH]), t1[g][:])
                      zg_tiles.append(zgt)
                  first = True
                  for g in range(gpt, GRP):
                      zvt = zv_pool.tile([128, GRP, CH], bf16, tag="zv")
                      nc.vector.tensor_mul(
                          zvt[:], xin.unsqueeze(1)
                          .broadcast_to([128, GRP, CH]), t1[g][:])
                      for j in range(GRP):
                          nc.tensor.matmul(ps[:], w_t[:, g * GRP + j, :],
                                           zvt[:, j, :], start=first,
                                           stop=False)
                          first = False
                  for g in range(gpt):
                      for j in range(GRP):
                          last = (g == gpt - 1) and (j == GRP - 1)
                          nc.tensor.matmul(ps[:], w_t[:, g * GRP + j, :],
                                           zg_tiles[g][:, j, :], start=False,
                                           stop=last)
                  xo = xc_pool.tile([128, CH], bf16, tag="xc")
                  nc.scalar.activation(xo[:], ps[:],
                                       mybir.ActivationFunctionType.Relu,
                                       bias=bias_ts[ps_i], scale=1.0)
                  nc.vector.tensor_reduce(
                      oacc[ps_i][:, c * (CH // D):(c + 1) * (CH // D)],
                      xo.rearrange("p (g d) -> p g d", d=D),
                      axis=mybir.AxisListType.X, op=mybir.AluOpType.add)
                  xin = xo

        for li in range(3):
            nc.sync.dma_start(out[li], oacc[li][:])

    nc.finalize()
    return nc


def _pack_weights(W0, b0, W1, b1, W2, b2):
    # Layer 0: fold symmetric pairs. Pair order: np.triu_indices(64).
    hh, ff = np.triu_indices(F)                     # 2080 pairs, h <= f
    W0r = np.asarray(W0, np.float32).reshape(O, F, F)
    W0sym = W0r[:, hh, ff] + np.where(hh != ff, 1.0, 0.0) * W0r[:, ff, hh]
    W0p = np.zeros((O, K0T * 128), np.float32)
    W0p[:, :NP] = W0sym                             # [o, k-row]
    w0p = np.ascontiguousarray(
        W0p.reshape(O, K0T, 128).transpose(2, 1, 0)).astype(BF16)
    w1p = np.ascontiguousarray(
        np.asarray(W1, np.float32).reshape(O, 128, F)
        .transpose(1, 2, 0)).astype(BF16)           # [h, f, o]
    w2p = np.ascontiguousarray(
        np.asarray(W2, np.float32).reshape(O, 128, F)
        .transpose(1, 2, 0)).astype(BF16)
    return {
        "w0": w0p, "w1": w1p, "w2": w2p,
        "b0": np.asarray(b0, np.float32).reshape(128, 1),
        "b1": np.asarray(b1, np.float32).reshape(128, 1),
        "b2": np.asarray(b2, np.float32).reshape(128, 1),
    }


def make_in_maps(input, W0, b0, W1, b1, W2, b2):
    shared = _pack_weights(W0, b0, W1, b1, W2, b2)
    hh, ff = np.triu_indices(F)
    in_maps = []
    inp_np = np.asarray(input)
    for c in range(NCORES):
        shard = inp_np[c * BL:(c + 1) * BL]          # [BL, F, D]
        IN = np.ascontiguousarray(
            shard.transpose(1, 0, 2).reshape(F, N)).astype(BF16)
        INf = IN.astype(np.float32)
        # z0 rows: products of the symmetric pairs, bf16 (same values the
        # device would produce from bf16 inputs)
        z0r = np.zeros((K0T * 128, N), BF16)
        z0r[:NP] = (INf[hh] * INf[ff]).astype(BF16)
        z0p = np.ascontiguousarray(
            z0r.reshape(K0T, 128, NCH, CH).transpose(2, 1, 0, 3))
        # broadcast-DMA source: ins[c, g, j, n] = IN[8g+j, ns+n]
        insp = np.ascontiguousarray(
            IN.reshape(GRP, GRP, NCH, CH).transpose(2, 0, 1, 3))
        in_maps.append({"ins": insp, "z0": z0p, **shared})
    return in_maps


def gather_out(results):
    # per-core out [3, 128, BL] -> full [B, 384]
    return np.concatenate(
        [np.asarray(r["out"], np.float32).transpose(2, 0, 1).reshape(BL, 3 * O)
         for r in results], axis=0)


def kernel(input, W0, b0, W1, b1, W2, b2):
    if "nc" not in _cache:
        _cache["nc"] = _build_program()
    nc = _cache["nc"]
    in_maps = make_in_maps(input, W0, b0, W1, b1, W2, b2)
    res = run_bass_kernel_spmd(nc, in_maps, list(range(NCORES)))
    return gather_out(res.results)


# revision 22
# speedup vs baseline: 1.4131x; 1.4131x over previous
"""CIN (xDeepFM) 3-layer kernel for Trainium2, 8-core data parallel.

Math (per layer l, with IN = input viewed [F=64, n] and X = previous
activation [H, n], n = (b, d) flattened):
    pre[o, n] = sum_{h, f} Wl[o, h, f] * X[h, n] * IN[f, n]
    Xnext = relu(pre + bl);  out_l[o, b] = sum_d Xnext[o, (b, d)]

Device strategy per core (64 batches, n = 0..2047):
  - Layer 0: z0[(h,f), n] = IN[h,n]*IN[f,n] is symmetric in (h,f), so W0 is
    folded to upper-triangle form on host (K 4096 -> 2080, padded 2176) and
    the z0 rows themselves are host-packed and DMA'd in (they are a pure
    function of the input, like im2col). No on-device elementwise work for
    layer 0, and only 17 K=128 matmul tiles.
  - Tables t[p, j, n] = IN[8g+j, n] are partition-broadcast by the DMA
    itself (stride-0 source over partitions): HBM reads are ~64KB instead
    of 33MB; only the SBUF-write side pays.
  - Layers 1/2: z k-tiles (8 f-rows x 128 h) built by elementwise
    tensor_mul: 7 tiles per layer on VectorE (2 elem/cycle bf16), 1 tile on
    GpSimd. The GpSimd tile is issued first after each relu (it is ~4x
    slower per tile) and its matmuls are accumulated last.
  - Matmuls in bf16 accumulate over f into one PSUM bank [o=128, 512].
  - ScalarE applies bias+ReLU straight from PSUM (per-partition bias),
    producing the next layer's X in its natural layout. No transposes.
  - VectorE grouped reduce sums over d (innermost 32) for the output.
"""

import numpy as np
import ml_dtypes

import concourse.bass as bass
import concourse.bacc as bacc
import concourse.tile as tile
import concourse.mybir as mybir
from concourse.bass_utils import run_bass_kernel_spmd

BF16 = ml_dtypes.bfloat16

B, F, D = 512, 64, 32
NCORES = 8
BL = B // NCORES          # 64 batches per core
N = BL * D                # 2048 columns per core
CH = 512                  # chunk width (columns)
NCH = N // CH             # 4 chunks
O = 128                   # out channels per layer
GRP = 8                   # f-rows per z tile
NP = 2080                 # symmetric (h<=f) pairs for layer 0
K0T = 17                  # ceil(2080/128) k-tiles for layer 0
GPT = 0                   # z tiles per layer built on GpSimd (of 8)
bf16 = mybir.dt.bfloat16
f32 = mybir.dt.float32

_cache = {}


def _build_program(bench_repeat=None, gpt=GPT, tables="dram", reduce_eng="mix",
                   unroll=1):
    assert reduce_eng in ("dve", "act", "mix")
    from contextlib import ExitStack, nullcontext

    nc = bacc.Bacc("TRN2")
    ins = nc.declare_dram_parameter("ins", [NCH, GRP, GRP, CH], bf16,
                                    isOutput=False)
    if tables == "dram":
        tab1 = nc.declare_dram_parameter("tab1", [NCH, GRP, 128, GRP, CH],
                                         bf16, isOutput=False)
    z0 = nc.declare_dram_parameter("z0", [NCH, 128, K0T, CH], bf16,
                                   isOutput=False)
    w0 = nc.declare_dram_parameter("w0", [128, K0T, 128], bf16, isOutput=False)
    w1 = nc.declare_dram_parameter("w1", [128, 64, 128], bf16, isOutput=False)
    w2 = nc.declare_dram_parameter("w2", [128, 64, 128], bf16, isOutput=False)
    b0 = nc.declare_dram_parameter("b0", [128, 1], f32, isOutput=False)
    b1 = nc.declare_dram_parameter("b1", [128, 1], f32, isOutput=False)
    b2 = nc.declare_dram_parameter("b2", [128, 1], f32, isOutput=False)
    out = nc.declare_dram_parameter("out", [3, 128, BL], f32, isOutput=True)

    with tile.TileContext(nc) as tc, ExitStack() as ctx:
        wpool = ctx.enter_context(tc.tile_pool(name="w", bufs=1))
        z0pool = ctx.enter_context(tc.tile_pool(name="z0", bufs=2))
        xc_pool = ctx.enter_context(tc.tile_pool(name="xc", bufs=4))
        tabs = ctx.enter_context(tc.tile_pool(name="tabs", bufs=11))
        zv_pool = ctx.enter_context(tc.tile_pool(name="zv", bufs=3))
        zg_pool = ctx.enter_context(tc.tile_pool(name="zg", bufs=2))
        opool = ctx.enter_context(tc.tile_pool(name="oacc", bufs=1))
        pspool = ctx.enter_context(tc.tile_pool(name="ps", bufs=4, space="PSUM"))

        # resident weights [p=k-row, f-slot, o] — on the ACT HWDGE queue so
        # the SP queue starts streaming tables at t=0
        w0_t = wpool.tile([128, K0T, 128], bf16)
        nc.scalar.dma_start(w0_t[:], w0[:])
        bias_ts = []
        for bi, bp in enumerate((b0, b1, b2)):
            b_t = wpool.tile([128, 1], f32, name=f"bias{bi}", tag=f"bias{bi}")
            nc.scalar.dma_start(b_t[:], bp[:])
            bias_ts.append(b_t)
        w1_t = wpool.tile([128, 64, 128], bf16)
        nc.scalar.dma_start(w1_t[:], w1[:])
        w2_t = wpool.tile([128, 64, 128], bf16)
        nc.scalar.dma_start(w2_t[:], w2[:])

        oacc = [opool.tile([128, BL], f32, name=f"oacc{i}", tag=f"oacc{i}")
                for i in range(3)]
        scr_pool = ctx.enter_context(tc.tile_pool(name="scr", bufs=2))
        zeros_t = wpool.tile([128, CH], bf16, name="zeros", tag="zeros")
        nc.vector.memset(zeros_t[:], 0.0)

        def reduce_to(oacc_i, c, xt):
            use_act = (reduce_eng == "act" or
                       (reduce_eng == "mix" and c < NCH - 1))
            if use_act:
                # d-sums on the (mostly idle) scalar engine via accum_out,
                # off the relu->tensor_mul critical path
                for i in range(CH // D):
                    scr = scr_pool.tile([128, D], bf16, tag="scr")
                    nc.scalar.activation(
                        scr[:], xt[:, i * D:(i + 1) * D],
                        mybir.ActivationFunctionType.Copy,
                        accum_out=oacc[oacc_i][:, c * (CH // D) + i:
                                               c * (CH // D) + i + 1])
                return
            nc.vector.tensor_reduce(
                oacc[oacc_i][:, c * (CH // D):(c + 1) * (CH // D)],
                xt.rearrange("p (g d) -> p g d", d=D),
                axis=mybir.AxisListType.X, op=mybir.AluOpType.add)

        def emit_stream(c):
            # z0 on the (otherwise idle) gpsimd SWDGE queue so it is neither
            # head-of-line blocked behind the bulk table DMAs on SP nor
            # blocking the relu chain on the ACT queue
            z0_t = z0pool.tile([128, K0T, CH], bf16, tag="z0")
            if c == 0:
                # split so layer 0 of the first chunk starts ~3us earlier
                nc.gpsimd.dma_start(z0_t[:, 0:9], z0[c, :, 0:9])
                nc.gpsimd.dma_start(z0_t[:, 9:K0T], z0[c, :, 9:K0T])
            else:
                nc.gpsimd.dma_start(z0_t[:], z0[c])
            t1 = []
            for g in range(GRP):
                s = tabs.tile([128, GRP, CH], bf16, tag="tab")
                if tables == "dram":
                    nc.sync.dma_start(s[:], tab1[c, g])
                else:
                    nc.sync.dma_start(
                        s[:],
                        ins[c, g].unsqueeze(0).broadcast_to([128, GRP, CH]))
                t1.append(s)
            return z0_t, t1

        def emit_l0(c, z0_t):
            # layer 0: 17 K=128 tiles from the host-packed symmetric z0
            ps0 = pspool.tile([128, CH], f32)
            for s in range(K0T):
                nc.tensor.matmul(ps0[:], w0_t[:, s, :], z0_t[:, s, :],
                                 start=(s == 0), stop=(s == K0T - 1))
            x1c = xc_pool.tile([128, CH], bf16, tag="xc")
            if c == 0:
                # chunk 0: relu on DVE so it does not queue behind the w1/w2
                # weight DMAs on the ACT queue at kernel start
                nc.vector.scalar_tensor_tensor(
                    x1c[:], ps0[:], bias_ts[0][:], zeros_t[:],
                    mybir.AluOpType.add, mybir.AluOpType.max)
            else:
                nc.scalar.activation(x1c[:], ps0[:],
                                     mybir.ActivationFunctionType.Relu,
                                     bias=bias_ts[0], scale=1.0)
            reduce_to(0, c, x1c)
            return x1c

        def emit_layer(c, li, xin, t1):
            w_t, ps_i = ((w1_t, 1), (w2_t, 2))[li - 1]
            ps = pspool.tile([128, CH], f32)
            zg_tiles = []
            for g in range(gpt):
                zgt = zg_pool.tile([128, GRP, CH], bf16, tag="zg")
                nc.gpsimd.tensor_mul(
                    zgt[:], xin.unsqueeze(1)
                    .broadcast_to([128, GRP, CH]), t1[g][:])
                zg_tiles.append(zgt)
            first = True
            for g in range(gpt, GRP):
                halves = ((0, GRP),) if g < GRP - 1 else \
                    ((0, GRP // 2), (GRP // 2, GRP))
                for (j0, j1) in halves:
                    nj = j1 - j0
                    zvt = zv_pool.tile([128, nj, CH], bf16,
                                       tag="zvh" if nj < GRP else "zv")
                    nc.vector.tensor_mul(
                        zvt[:], xin.unsqueeze(1)
                        .broadcast_to([128, nj, CH]), t1[g][:, j0:j1])
                    for j in range(j0, j1):
                        last = (gpt == 0) and (g == GRP - 1) and (j == GRP - 1)
                        nc.tensor.matmul(ps[:], w_t[:, g * GRP + j, :],
                                         zvt[:, j - j0, :], start=first,
                                         stop=last)
                        first = False
            for g in range(gpt):
                for j in range(GRP):
                    last = (g == gpt - 1) and (j == GRP - 1)
                    nc.tensor.matmul(ps[:], w_t[:, g * GRP + j, :],
                                     zg_tiles[g][:, j, :], start=False,
                                     stop=last)
            xo = xc_pool.tile([128, CH], bf16, tag="xc")
            nc.scalar.activation(xo[:], ps[:],
                                 mybir.ActivationFunctionType.Relu,
                                 bias=bias_ts[ps_i], scale=1.0)
            reduce_to(ps_i, c, xo)
            return xo

        # Software pipeline: chunk c+1's stream + layer 0 are emitted between
        # chunk c's layers 1 and 2, filling the relu-boundary stalls on PE.
        loop_cm = tc.For_i(0, bench_repeat, 1) if bench_repeat else nullcontext()
        with loop_cm:
          for _u in range(unroll):
            z0_t, t1 = emit_stream(0)
            x1 = emit_l0(0, z0_t)
            for c in range(NCH):
                nxt = None
                if c + 1 < NCH:
                    nxt = emit_stream(c + 1)
                x2 = emit_layer(c, 1, x1, t1)
                if nxt is not None:
                    x1 = emit_l0(c + 1, nxt[0])
                emit_layer(c, 2, x2, t1)
                if nxt is not None:
                    t1 = nxt[1]

        for li in range(3):
            nc.sync.dma_start(out[li], oacc[li][:])

    nc.finalize()
    return nc


def _pack_weights(W0, b0, W1, b1, W2, b2):
    # Layer 0: fold symmetric pairs. Pair order: np.triu_indices(64).
    hh, ff = np.triu_indices(F)                     # 2080 pairs, h <= f
    W0r = np.asarray(W0, np.float32).reshape(O, F, F)
    W0sym = W0r[:, hh, ff] + np.where(hh != ff, 1.0, 0.0) * W0r[:, ff, hh]
    W0p = np.zeros((O, K0T * 128), np.float32)
    W0p[:, :NP] = W0sym                             # [o, k-row]
    w0p = np.ascontiguousarray(
        W0p.reshape(O, K0T, 128).transpose(2, 1, 0)).astype(BF16)
    w1p = np.ascontiguousarray(
        np.asarray(W1, np.float32).reshape(O, 128, F)
        .transpose(1, 2, 0)).astype(BF16)           # [h, f, o]
    w2p = np.ascontiguousarray(
        np.asarray(W2, np.float32).reshape(O, 128, F)
        .transpose(1, 2, 0)).astype(BF16)
    return {
        "w0": w0p, "w1": w1p, "w2": w2p,
        "b0": np.asarray(b0, np.float32).reshape(128, 1),
        "b1": np.asarray(b1, np.float32).reshape(128, 1),
        "b2": np.asarray(b2, np.float32).reshape(128, 1),
    }


def make_in_maps(input, W0, b0, W1, b1, W2, b2, tables="dram"):
    shared = _pack_weights(W0, b0, W1, b1, W2, b2)
    hh, ff = np.triu_indices(F)
    in_maps = []
    inp_np = np.asarray(input)
    for c in range(NCORES):
        shard = inp_np[c * BL:(c + 1) * BL]          # [BL, F, D]
        IN = np.ascontiguousarray(
            shard.transpose(1, 0, 2).reshape(F, N)).astype(BF16)
        INf = IN.astype(np.float32)
        # z0 rows: products of the symmetric pairs, bf16 (same values the
        # device would produce from bf16 inputs)
        z0r = np.zeros((K0T * 128, N), BF16)
        z0r[:NP] = (INf[hh] * INf[ff]).astype(BF16)
        z0p = np.ascontiguousarray(
            z0r.reshape(K0T, 128, NCH, CH).transpose(2, 1, 0, 3))
        # broadcast-DMA source: ins[c, g, j, n] = IN[8g+j, ns+n]
        insp = np.ascontiguousarray(
            IN.reshape(GRP, GRP, NCH, CH).transpose(2, 0, 1, 3))
        m = {"ins": insp, "z0": z0p, **shared}
        if tables == "dram":
            # fully-replicated tables: tab1[c, g, p, j, n] = IN[8g+j, ns+n]
            t1r = np.transpose(IN.reshape(GRP, GRP, NCH, CH), (2, 0, 1, 3))
            t1a = np.empty((NCH, GRP, 128, GRP, CH), BF16)
            t1a[:, :] = t1r[:, :, None, :, :]
            m["tab1"] = t1a
        in_maps.append(m)
    return in_maps


def gather_out(results):
    # per-core out [3, 128, BL] -> full [B, 384]
    return np.concatenate(
        [np.asarray(r["out"], np.float32).transpose(2, 0, 1).reshape(BL, 3 * O)
         for r in results], axis=0)


def kernel(input, W0, b0, W1, b1, W2, b2):
    if "nc" not in _cache:
        _cache["nc"] = _build_program()
    nc = _cache["nc"]
    in_maps = make_in_maps(input, W0, b0, W1, b1, W2, b2)
    res = run_bass_kernel_spmd(nc, in_maps, list(range(NCORES)))
    return gather_out(res.results)


# revision 25
# speedup vs baseline: 1.5885x; 1.1241x over previous
"""CIN (xDeepFM) 3-layer kernel for Trainium2, 8-core data parallel.

Math (per layer l, with IN = input viewed [F=64, n] and X = previous
activation [H, n], n = (b, d) flattened):
    pre[o, n] = sum_{h, f} Wl[o, h, f] * X[h, n] * IN[f, n]
    Xnext = relu(pre + bl);  out_l[o, b] = sum_d Xnext[o, (b, d)]

Device strategy per core (64 batches, n = 0..2047):
  - Layer 0: z0[(h,f), n] = IN[h,n]*IN[f,n] is symmetric in (h,f), so W0 is
    folded to upper-triangle form on host (K 4096 -> 2080, padded 2176) and
    the z0 rows themselves are host-packed and DMA'd in (they are a pure
    function of the input, like im2col). No on-device elementwise work for
    layer 0, and only 17 K=128 matmul tiles.
  - Tables t[p, j, n] = IN[8g+j, n] are partition-broadcast by the DMA
    itself (stride-0 source over partitions): HBM reads are ~64KB instead
    of 33MB; only the SBUF-write side pays.
  - Layers 1/2: z k-tiles (8 f-rows x 128 h) built by elementwise
    tensor_mul: 7 tiles per layer on VectorE (2 elem/cycle bf16), 1 tile on
    GpSimd. The GpSimd tile is issued first after each relu (it is ~4x
    slower per tile) and its matmuls are accumulated last.
  - Matmuls in bf16 accumulate over f into one PSUM bank [o=128, 512].
  - ScalarE applies bias+ReLU straight from PSUM (per-partition bias),
    producing the next layer's X in its natural layout. No transposes.
  - VectorE grouped reduce sums over d (innermost 32) for the output.
"""

import numpy as np
import ml_dtypes

import concourse.bass as bass
import concourse.bacc as bacc
import concourse.tile as tile
import concourse.mybir as mybir
from concourse.bass_utils import run_bass_kernel_spmd

BF16 = ml_dtypes.bfloat16

B, F, D = 512, 64, 32
NCORES = 8
BL = B // NCORES          # 64 batches per core
N = BL * D                # 2048 columns per core
CH = 512                  # chunk width (columns)
NCH = N // CH             # 4 chunks
O = 128                   # out channels per layer
GRP = 8                   # f-rows per z tile
NP = 2080                 # symmetric (h<=f) pairs for layer 0
K0T = 17                  # ceil(2080/128) k-tiles for layer 0
GPT = 0                   # z tiles per layer built on GpSimd (of 8)
bf16 = mybir.dt.bfloat16
f32 = mybir.dt.float32

_cache = {}


def _build_program(bench_repeat=None, gpt=GPT, tables="dram", reduce_eng="mix",
                   unroll=1):
    assert reduce_eng in ("dve", "act", "mix")
    from contextlib import ExitStack, nullcontext

    nc = bacc.Bacc("TRN2")
    ins = nc.declare_dram_parameter("ins", [NCH, GRP, GRP, CH], bf16,
                                    isOutput=False)
    if tables == "dram":
        tab1 = nc.declare_dram_parameter("tab1", [NCH, GRP, 128, GRP, CH],
                                         bf16, isOutput=False)
    z0 = nc.declare_dram_parameter("z0", [NCH, 128, K0T, CH], bf16,
                                   isOutput=False)
    w0 = nc.declare_dram_parameter("w0", [128, K0T, 128], bf16, isOutput=False)
    w1 = nc.declare_dram_parameter("w1", [128, 64, 128], bf16, isOutput=False)
    w2 = nc.declare_dram_parameter("w2", [128, 64, 128], bf16, isOutput=False)
    b0 = nc.declare_dram_parameter("b0", [128, 1], f32, isOutput=False)
    b1 = nc.declare_dram_parameter("b1", [128, 1], f32, isOutput=False)
    b2 = nc.declare_dram_parameter("b2", [128, 1], f32, isOutput=False)
    out = nc.declare_dram_parameter("out", [3, 128, BL], f32, isOutput=True)

    with tile.TileContext(nc) as tc, ExitStack() as ctx:
        wpool = ctx.enter_context(tc.tile_pool(name="w", bufs=1))
        z0pool = ctx.enter_context(tc.tile_pool(name="z0", bufs=2))
        xc_pool = ctx.enter_context(tc.tile_pool(name="xc", bufs=6))
        tabs = ctx.enter_context(tc.tile_pool(name="tabs", bufs=11))
        zv_pool = ctx.enter_context(tc.tile_pool(name="zv", bufs=3))
        zg_pool = ctx.enter_context(tc.tile_pool(name="zg", bufs=2))
        opool = ctx.enter_context(tc.tile_pool(name="oacc", bufs=1))
        pspool = ctx.enter_context(tc.tile_pool(name="ps", bufs=4, space="PSUM"))

        # resident weights [p=k-row, f-slot, o] — on the ACT HWDGE queue so
        # the SP queue starts streaming tables at t=0
        w0_t = wpool.tile([128, K0T, 128], bf16)
        nc.scalar.dma_start(w0_t[:], w0[:])
        bias_ts = []
        for bi, bp in enumerate((b0, b1, b2)):
            b_t = wpool.tile([128, 1], f32, name=f"bias{bi}", tag=f"bias{bi}")
            nc.scalar.dma_start(b_t[:], bp[:])
            bias_ts.append(b_t)
        w1_t = wpool.tile([128, 64, 128], bf16)
        nc.scalar.dma_start(w1_t[:], w1[:])
        w2_t = wpool.tile([128, 64, 128], bf16)
        nc.scalar.dma_start(w2_t[:], w2[:])

        oacc = [opool.tile([128, BL], f32, name=f"oacc{i}", tag=f"oacc{i}")
                for i in range(3)]
        scr_pool = ctx.enter_context(tc.tile_pool(name="scr", bufs=2))
        zeros_t = wpool.tile([128, CH], bf16, name="zeros", tag="zeros")
        nc.vector.memset(zeros_t[:], 0.0)

        def reduce_to(oacc_i, c, xt):
            use_act = (reduce_eng == "act" or
                       (reduce_eng == "mix" and c < NCH - 1))
            if use_act:
                # d-sums on the (mostly idle) scalar engine via accum_out,
                # off the relu->tensor_mul critical path
                for i in range(CH // D):
                    scr = scr_pool.tile([128, D], bf16, tag="scr")
                    nc.scalar.activation(
                        scr[:], xt[:, i * D:(i + 1) * D],
                        mybir.ActivationFunctionType.Copy,
                        accum_out=oacc[oacc_i][:, c * (CH // D) + i:
                                               c * (CH // D) + i + 1])
                return
            nc.vector.tensor_reduce(
                oacc[oacc_i][:, c * (CH // D):(c + 1) * (CH // D)],
                xt.rearrange("p (g d) -> p g d", d=D),
                axis=mybir.AxisListType.X, op=mybir.AluOpType.add)

        def emit_stream(c):
            # z0 on the (otherwise idle) gpsimd SWDGE queue so it is neither
            # head-of-line blocked behind the bulk table DMAs on SP nor
            # blocking the relu chain on the ACT queue
            z0_t = z0pool.tile([128, K0T, CH], bf16, tag="z0")
            if c == 0:
                # split so layer 0 of the first chunk starts ~2us earlier
                for (a, b) in ((0, 4), (4, 9), (9, 13), (13, K0T)):
                    nc.gpsimd.dma_start(z0_t[:, a:b], z0[c, :, a:b])
            else:
                nc.gpsimd.dma_start(z0_t[:], z0[c])
            t1 = []
            for g in range(GRP):
                s = tabs.tile([128, GRP, CH], bf16, tag="tab")
                if tables == "dram":
                    nc.sync.dma_start(s[:], tab1[c, g])
                else:
                    nc.sync.dma_start(
                        s[:],
                        ins[c, g].unsqueeze(0).broadcast_to([128, GRP, CH]))
                t1.append(s)
            return z0_t, t1

        def emit_l0(c, z0_t):
            # layer 0: 17 K=128 tiles from the host-packed symmetric z0
            ps0 = pspool.tile([128, CH], f32)
            for s in range(K0T):
                nc.tensor.matmul(ps0[:], w0_t[:, s, :], z0_t[:, s, :],
                                 start=(s == 0), stop=(s == K0T - 1))
            x1c = xc_pool.tile([128, CH], bf16, tag="xc")
            if c == 0:
                # chunk 0: relu on DVE so it does not queue behind the w1/w2
                # weight DMAs on the ACT queue at kernel start
                nc.vector.scalar_tensor_tensor(
                    x1c[:], ps0[:], bias_ts[0][:], zeros_t[:],
                    mybir.AluOpType.add, mybir.AluOpType.max)
            else:
                nc.scalar.activation(x1c[:], ps0[:],
                                     mybir.ActivationFunctionType.Relu,
                                     bias=bias_ts[0], scale=1.0)
            reduce_to(0, c, x1c)
            return x1c

        def emit_layer(c, li, xin, t1):
            w_t, ps_i = ((w1_t, 1), (w2_t, 2))[li - 1]
            ps = pspool.tile([128, CH], f32)
            zg_tiles = []
            for g in range(gpt):
                zgt = zg_pool.tile([128, GRP, CH], bf16, tag="zg")
                nc.gpsimd.tensor_mul(
                    zgt[:], xin.unsqueeze(1)
                    .broadcast_to([128, GRP, CH]), t1[g][:])
                zg_tiles.append(zgt)
            first = True
            for g in range(gpt, GRP):
                halves = ((0, GRP),) if g < GRP - 1 else \
                    ((0, 4), (4, 6), (6, 8))
                for (j0, j1) in halves:
                    nj = j1 - j0
                    zvt = zv_pool.tile([128, nj, CH], bf16,
                                       tag="zvh" if nj < GRP else "zv")
                    nc.vector.tensor_mul(
                        zvt[:], xin.unsqueeze(1)
                        .broadcast_to([128, nj, CH]), t1[g][:, j0:j1])
                    for j in range(j0, j1):
                        last = (gpt == 0) and (g == GRP - 1) and (j == GRP - 1)
                        nc.tensor.matmul(ps[:], w_t[:, g * GRP + j, :],
                                         zvt[:, j - j0, :], start=first,
                                         stop=last)
                        first = False
            for g in range(gpt):
                for j in range(GRP):
                    last = (g == gpt - 1) and (j == GRP - 1)
                    nc.tensor.matmul(ps[:], w_t[:, g * GRP + j, :],
                                     zg_tiles[g][:, j, :], start=False,
                                     stop=last)
            xo = xc_pool.tile([128, CH], bf16, tag="xc")
            nc.scalar.activation(xo[:], ps[:],
                                 mybir.ActivationFunctionType.Relu,
                                 bias=bias_ts[ps_i], scale=1.0)
            reduce_to(ps_i, c, xo)
            return xo

        # Software pipeline: chunk c+1's stream + layer 0 are emitted between
        # chunk c's layers 1 and 2, filling the relu-boundary stalls on PE.
        loop_cm = tc.For_i(0, bench_repeat, 1) if bench_repeat else nullcontext()
        with loop_cm:
          for _u in range(unroll):
            z0_t, t1 = emit_stream(0)
            x1 = emit_l0(0, z0_t)
            for c in range(NCH):
                nxt = None
                if c + 1 < NCH:
                    nxt = emit_stream(c + 1)
                x2 = emit_layer(c, 1, x1, t1)
                if nxt is not None:
                    x1 = emit_l0(c + 1, nxt[0])
                emit_layer(c, 2, x2, t1)
                if nxt is not None:
                    t1 = nxt[1]

        for li in range(3):
            nc.sync.dma_start(out[li], oacc[li][:])

    nc.finalize()
    return nc


def _pack_weights(W0, b0, W1, b1, W2, b2):
    # Layer 0: fold symmetric pairs. Pair order: np.triu_indices(64).
    hh, ff = np.triu_indices(F)                     # 2080 pairs, h <= f
    W0r = np.asarray(W0, np.float32).reshape(O, F, F)
    W0sym = W0r[:, hh, ff] + np.where(hh != ff, 1.0, 0.0) * W0r[:, ff, hh]
    W0p = np.zeros((O, K0T * 128), np.float32)
    W0p[:, :NP] = W0sym                             # [o, k-row]
    w0p = np.ascontiguousarray(
        W0p.reshape(O, K0T, 128).transpose(2, 1, 0)).astype(BF16)
    w1p = np.ascontiguousarray(
        np.asarray(W1, np.float32).reshape(O, 128, F)
        .transpose(1, 2, 0)).astype(BF16)           # [h, f, o]
    w2p = np.ascontiguousarray(
        np.asarray(W2, np.float32).reshape(O, 128, F)
        .transpose(1, 2, 0)).astype(BF16)
    return {
        "w0": w0p, "w1": w1p, "w2": w2p,
        "b0": np.asarray(b0, np.float32).reshape(128, 1),
        "b1": np.asarray(b1, np.float32).reshape(128, 1),
        "b2": np.asarray(b2, np.float32).reshape(128, 1),
    }


def make_in_maps(input, W0, b0, W1, b1, W2, b2, tables="dram"):
    shared = _pack_weights(W0, b0, W1, b1, W2, b2)
    hh, ff = np.triu_indices(F)
    in_maps = []
    inp_np = np.asarray(input)
    for c in range(NCORES):
        shard = inp_np[c * BL:(c + 1) * BL]          # [BL, F, D]
        IN = np.ascontiguousarray(
            shard.transpose(1, 0, 2).reshape(F, N)).astype(BF16)
        INf = IN.astype(np.float32)
        # z0 rows: products of the symmetric pairs, bf16 (same values the
        # device would produce from bf16 inputs)
        z0r = np.zeros((K0T * 128, N), BF16)
        z0r[:NP] = (INf[hh] * INf[ff]).astype(BF16)
        z0p = np.ascontiguousarray(
            z0r.reshape(K0T, 128, NCH, CH).transpose(2, 1, 0, 3))
        # broadcast-DMA source: ins[c, g, j, n] = IN[8g+j, ns+n]
        insp = np.ascontiguousarray(
            IN.reshape(GRP, GRP, NCH, CH).transpose(2, 0, 1, 3))
        m = {"ins": insp, "z0": z0p, **shared}
        if tables == "dram":
            # fully-replicated tables: tab1[c, g, p, j, n] = IN[8g+j, ns+n]
            t1r = np.transpose(IN.reshape(GRP, GRP, NCH, CH), (2, 0, 1, 3))
            t1a = np.empty((NCH, GRP, 128, GRP, CH), BF16)
            t1a[:, :] = t1r[:, :, None, :, :]
            m["tab1"] = t1a
        in_maps.append(m)
    return in_maps


def gather_out(results):
    # per-core out [3, 128, BL] -> full [B, 384]
    return np.concatenate(
        [np.asarray(r["out"], np.float32).transpose(2, 0, 1).reshape(BL, 3 * O)
         for r in results], axis=0)


def kernel(input, W0, b0, W1, b1, W2, b2):
    if "nc" not in _cache:
        _cache["nc"] = _build_program()
    nc = _cache["nc"]
    in_maps = make_in_maps(input, W0, b0, W1, b1, W2, b2)
    res = run_bass_kernel_spmd(nc, in_maps, list(range(NCORES)))
    return gather_out(res.results)


# revision 27
# speedup vs baseline: 1.6120x; 1.0148x over previous
"""CIN (xDeepFM) 3-layer kernel for Trainium2, 8-core data parallel.

Math (per layer l, with IN = input viewed [F=64, n] and X = previous
activation [H, n], n = (b, d) flattened):
    pre[o, n] = sum_{h, f} Wl[o, h, f] * X[h, n] * IN[f, n]
    Xnext = relu(pre + bl);  out_l[o, b] = sum_d Xnext[o, (b, d)]

Device strategy per core (64 batches, n = 0..2047):
  - Layer 0: z0[(h,f), n] = IN[h,n]*IN[f,n] is symmetric in (h,f), so W0 is
    folded to upper-triangle form on host (K 4096 -> 2080, padded 2176) and
    the z0 rows themselves are host-packed and DMA'd in (they are a pure
    function of the input, like im2col). No on-device elementwise work for
    layer 0, and only 17 K=128 matmul tiles.
  - Tables t[p, j, n] = IN[8g+j, n] are partition-broadcast by the DMA
    itself (stride-0 source over partitions): HBM reads are ~64KB instead
    of 33MB; only the SBUF-write side pays.
  - Layers 1/2: z k-tiles (8 f-rows x 128 h) built by elementwise
    tensor_mul: 7 tiles per layer on VectorE (2 elem/cycle bf16), 1 tile on
    GpSimd. The GpSimd tile is issued first after each relu (it is ~4x
    slower per tile) and its matmuls are accumulated last.
  - Matmuls in bf16 accumulate over f into one PSUM bank [o=128, 512].
  - ScalarE applies bias+ReLU straight from PSUM (per-partition bias),
    producing the next layer's X in its natural layout. No transposes.
  - VectorE grouped reduce sums over d (innermost 32) for the output.
"""

import numpy as np
import ml_dtypes

import concourse.bass as bass
import concourse.bacc as bacc
import concourse.tile as tile
import concourse.mybir as mybir
from concourse.bass_utils import run_bass_kernel_spmd

BF16 = ml_dtypes.bfloat16

B, F, D = 512, 64, 32
NCORES = 8
BL = B // NCORES          # 64 batches per core
N = BL * D                # 2048 columns per core
CH = 512                  # chunk width (columns)
NCH = N // CH             # 4 chunks
O = 128                   # out channels per layer
GRP = 8                   # f-rows per z tile
NP = 2080                 # symmetric (h<=f) pairs for layer 0
K0T = 17                  # ceil(2080/128) k-tiles for layer 0
GPT = 0                   # z tiles per layer built on GpSimd (of 8)
bf16 = mybir.dt.bfloat16
f32 = mybir.dt.float32

_cache = {}


def _build_program(bench_repeat=None, gpt=GPT, tables="dram", reduce_eng="mix",
                   unroll=1):
    assert reduce_eng in ("dve", "act", "mix")
    from contextlib import ExitStack, nullcontext

    nc = bacc.Bacc("TRN2")
    ins = nc.declare_dram_parameter("ins", [NCH, GRP, GRP, CH], bf16,
                                    isOutput=False)
    if tables == "dram":
        tab1 = nc.declare_dram_parameter("tab1", [NCH, GRP, 128, GRP, CH],
                                         bf16, isOutput=False)
    z0 = nc.declare_dram_parameter("z0", [NCH, 128, K0T, CH], bf16,
                                   isOutput=False)
    w0 = nc.declare_dram_parameter("w0", [128, K0T, 128], bf16, isOutput=False)
    w1 = nc.declare_dram_parameter("w1", [128, 64, 128], bf16, isOutput=False)
    w2 = nc.declare_dram_parameter("w2", [128, 64, 128], bf16, isOutput=False)
    b0 = nc.declare_dram_parameter("b0", [128, 1], f32, isOutput=False)
    b1 = nc.declare_dram_parameter("b1", [128, 1], f32, isOutput=False)
    b2 = nc.declare_dram_parameter("b2", [128, 1], f32, isOutput=False)
    out = nc.declare_dram_parameter("out", [3, 128, BL], f32, isOutput=True)

    with tile.TileContext(nc) as tc, ExitStack() as ctx:
        wpool = ctx.enter_context(tc.tile_pool(name="w", bufs=1))
        z0pool = ctx.enter_context(tc.tile_pool(name="z0", bufs=2))
        xc_pool = ctx.enter_context(tc.tile_pool(name="xc", bufs=6))
        tabs = ctx.enter_context(tc.tile_pool(name="tabs", bufs=11))
        zv_pool = ctx.enter_context(tc.tile_pool(name="zv", bufs=3))
        zg_pool = ctx.enter_context(tc.tile_pool(name="zg", bufs=2))
        opool = ctx.enter_context(tc.tile_pool(name="oacc", bufs=1))
        pspool = ctx.enter_context(tc.tile_pool(name="ps", bufs=4, space="PSUM"))

        # resident weights [p=k-row, f-slot, o] — on the ACT HWDGE queue so
        # the SP queue starts streaming tables at t=0
        w0_t = wpool.tile([128, K0T, 128], bf16)
        nc.scalar.dma_start(w0_t[:], w0[:])
        bias_ts = []
        for bi, bp in enumerate((b0, b1, b2)):
            b_t = wpool.tile([128, 1], f32, name=f"bias{bi}", tag=f"bias{bi}")
            nc.scalar.dma_start(b_t[:], bp[:])
            bias_ts.append(b_t)
        w1_t = wpool.tile([128, 64, 128], bf16)
        nc.scalar.dma_start(w1_t[:], w1[:])
        w2_t = wpool.tile([128, 64, 128], bf16)
        nc.scalar.dma_start(w2_t[:], w2[:])

        oacc = [opool.tile([128, BL], f32, name=f"oacc{i}", tag=f"oacc{i}")
                for i in range(3)]
        scr_pool = ctx.enter_context(tc.tile_pool(name="scr", bufs=2))
        zeros_t = wpool.tile([128, CH], bf16, name="zeros", tag="zeros")
        nc.vector.memset(zeros_t[:], 0.0)

        def reduce_to(oacc_i, c, xt):
            use_act = (reduce_eng == "act" or
                       (reduce_eng == "mix" and c < NCH - 1))
            if use_act:
                # d-sums on the (mostly idle) scalar engine via accum_out,
                # off the relu->tensor_mul critical path
                for i in range(CH // D):
                    scr = scr_pool.tile([128, D], bf16, tag="scr")
                    nc.scalar.activation(
                        scr[:], xt[:, i * D:(i + 1) * D],
                        mybir.ActivationFunctionType.Copy,
                        accum_out=oacc[oacc_i][:, c * (CH // D) + i:
                                               c * (CH // D) + i + 1])
                return
            nc.vector.tensor_reduce(
                oacc[oacc_i][:, c * (CH // D):(c + 1) * (CH // D)],
                xt.rearrange("p (g d) -> p g d", d=D),
                axis=mybir.AxisListType.X, op=mybir.AluOpType.add)

        def emit_stream(c):
            # z0 on the (otherwise idle) gpsimd SWDGE queue so it is neither
            # head-of-line blocked behind the bulk table DMAs on SP nor
            # blocking the relu chain on the ACT queue
            z0_t = z0pool.tile([128, K0T, CH], bf16, tag="z0")
            if c == 0:
                # split so layer 0 of the first chunk starts ~2us earlier
                for (a, b) in ((0, 4), (4, 9), (9, 13), (13, K0T)):
                    nc.gpsimd.dma_start(z0_t[:, a:b], z0[c, :, a:b])
            else:
                nc.gpsimd.dma_start(z0_t[:], z0[c])
            t1 = []
            for g in range(GRP):
                s = tabs.tile([128, GRP, CH], bf16, tag="tab")
                if tables == "dram":
                    nc.sync.dma_start(s[:], tab1[c, g])
                else:
                    nc.sync.dma_start(
                        s[:],
                        ins[c, g].unsqueeze(0).broadcast_to([128, GRP, CH]))
                t1.append(s)
            return z0_t, t1

        def emit_l0(c, z0_t):
            # layer 0: 17 K=128 tiles from the host-packed symmetric z0
            ps0 = pspool.tile([128, CH], f32)
            for s in range(K0T):
                nc.tensor.matmul(ps0[:], w0_t[:, s, :], z0_t[:, s, :],
                                 start=(s == 0), stop=(s == K0T - 1))
            x1c = xc_pool.tile([128, CH], bf16, tag="xc")
            if c == 0:
                # chunk 0: relu on DVE so it does not queue behind the w1/w2
                # weight DMAs on the ACT queue at kernel start
                nc.vector.scalar_tensor_tensor(
                    x1c[:], ps0[:], bias_ts[0][:], zeros_t[:],
                    mybir.AluOpType.add, mybir.AluOpType.max)
            else:
                nc.scalar.activation(x1c[:], ps0[:],
                                     mybir.ActivationFunctionType.Relu,
                                     bias=bias_ts[0], scale=1.0)
            reduce_to(0, c, x1c)
            return x1c

        def emit_layer(c, li, xin, t1):
            w_t, ps_i = ((w1_t, 1), (w2_t, 2))[li - 1]
            ps = pspool.tile([128, CH], f32)
            zg_tiles = []
            for g in range(gpt):
                zgt = zg_pool.tile([128, GRP, CH], bf16, tag="zg")
                nc.gpsimd.tensor_mul(
                    zgt[:], xin.unsqueeze(1)
                    .broadcast_to([128, GRP, CH]), t1[g][:])
                zg_tiles.append(zgt)
            first = True
            for g in range(gpt, GRP):
                halves = ((0, GRP),) if g < GRP - 1 else \
                    ((0, 4), (4, 6), (6, 8))
                for (j0, j1) in halves:
                    nj = j1 - j0
                    zvt = zv_pool.tile([128, nj, CH], bf16,
                                       tag="zvh" if nj < GRP else "zv")
                    nc.vector.tensor_mul(
                        zvt[:], xin.unsqueeze(1)
                        .broadcast_to([128, nj, CH]), t1[g][:, j0:j1])
                    for j in range(j0, j1):
                        last = (gpt == 0) and (g == GRP - 1) and (j == GRP - 1)
                        nc.tensor.matmul(ps[:], w_t[:, g * GRP + j, :],
                                         zvt[:, j - j0, :], start=first,
                                         stop=last)
                        first = False
            for g in range(gpt):
                for j in range(GRP):
                    last = (g == gpt - 1) and (j == GRP - 1)
                    nc.tensor.matmul(ps[:], w_t[:, g * GRP + j, :],
                                     zg_tiles[g][:, j, :], start=False,
                                     stop=last)
            xo = xc_pool.tile([128, CH], bf16, tag="xc")
            nc.scalar.activation(xo[:], ps[:],
                                 mybir.ActivationFunctionType.Relu,
                                 bias=bias_ts[ps_i], scale=1.0)
            reduce_to(ps_i, c, xo)
            return xo

        # Software pipeline: chunk c+1's stream + layer 0 are emitted between
        # chunk c's layers 1 and 2, filling the relu-boundary stalls on PE.
        loop_cm = tc.For_i(0, bench_repeat, 1) if bench_repeat else nullcontext()
        with loop_cm:
          for _u in range(unroll):
            z0_t, t1 = emit_stream(0)
            x1 = emit_l0(0, z0_t)
            for c in range(NCH):
                nxt = None
                if c + 1 < NCH:
                    nxt = emit_stream(c + 1)
                x2 = emit_layer(c, 1, x1, t1)
                if nxt is not None:
                    x1 = emit_l0(c + 1, nxt[0])
                emit_layer(c, 2, x2, t1)
                if nxt is not None:
                    t1 = nxt[1]

        for li in range(3):
            nc.sync.dma_start(out[li], oacc[li][:])

    nc.finalize()
    return nc


def _pack_weights(W0, b0, W1, b1, W2, b2):
    # Layer 0: fold symmetric pairs. Pair order: np.triu_indices(64).
    hh, ff = np.triu_indices(F)                     # 2080 pairs, h <= f
    W0r = np.asarray(W0, np.float32).reshape(O, F, F)
    W0sym = W0r[:, hh, ff] + np.where(hh != ff, 1.0, 0.0) * W0r[:, ff, hh]
    W0p = np.zeros((O, K0T * 128), np.float32)
    W0p[:, :NP] = W0sym                             # [o, k-row]
    w0p = np.ascontiguousarray(
        W0p.reshape(O, K0T, 128).transpose(2, 1, 0)).astype(BF16)
    w1p = np.ascontiguousarray(
        np.asarray(W1, np.float32).reshape(O, 128, F)
        .transpose(1, 2, 0)).astype(BF16)           # [h, f, o]
    w2p = np.ascontiguousarray(
        np.asarray(W2, np.float32).reshape(O, 128, F)
        .transpose(1, 2, 0)).astype(BF16)
    return {
        "w0": w0p, "w1": w1p, "w2": w2p,
        "b0": np.asarray(b0, np.float32).reshape(128, 1),
        "b1": np.asarray(b1, np.float32).reshape(128, 1),
        "b2": np.asarray(b2, np.float32).reshape(128, 1),
    }


def make_in_maps(input, W0, b0, W1, b1, W2, b2, tables="dram"):
    shared = _pack_weights(W0, b0, W1, b1, W2, b2)
    hh, ff = np.triu_indices(F)
    in_maps = []
    inp_np = np.asarray(input)
    for c in range(NCORES):
        shard = inp_np[c * BL:(c + 1) * BL]          # [BL, F, D]
        IN = np.ascontiguousarray(
            shard.transpose(1, 0, 2).reshape(F, N)).astype(BF16)
        INf = IN.astype(np.float32)
        # z0 rows: products of the symmetric pairs, bf16 (same values the
        # device would produce from bf16 inputs)
        z0r = np.zeros((K0T * 128, N), BF16)
        z0r[:NP] = (INf[hh] * INf[ff]).astype(BF16)
        z0p = np.ascontiguousarray(
            z0r.reshape(K0T, 128, NCH, CH).transpose(2, 1, 0, 3))
        # broadcast-DMA source: ins[c, g, j, n] = IN[8g+j, ns+n]
        insp = np.ascontiguousarray(
            IN.reshape(GRP, GRP, NCH, CH).transpose(2, 0, 1, 3))
        m = {"ins": insp, "z0": z0p, **shared}
        if tables == "dram":
            # fully-replicated tables: tab1[c, g, p, j, n] = IN[8g+j, ns+n]
            t1r = np.transpose(IN.reshape(GRP, GRP, NCH, CH), (2, 0, 1, 3))
            t1a = np.empty((NCH, GRP, 128, GRP, CH), BF16)
            t1a[:, :] = t1r[:, :, None, :, :]
            m["tab1"] = t1a
        in_maps.append(m)
    return in_maps


def gather_out(results):
    # per-core out [3, 128, BL] -> full [B, 384]
    return np.concatenate(
        [np.asarray(r["out"], np.float32).transpose(2, 0, 1).reshape(BL, 3 * O)
         for r in results], axis=0)


def kernel(input, W0, b0, W1, b1, W2, b2):
    if "nc" not in _cache:
        _cache["nc"] = _build_program()
    nc = _cache["nc"]
    in_maps = make_in_maps(input, W0, b0, W1, b1, W2, b2)
    res = run_bass_kernel_spmd(nc, in_maps, list(range(NCORES)))
    return gather_out(res.results)
